# revision 29
# baseline (speedup 1.0000x reference)
"""Depthwise-masked 3x3 conv (eye-masked dense conv) on 8 TRN2 NeuronCores.

Problem: x (2,16,256,64,64) fp32, W (256,256,3,3) fp32; the reference masks W
with eye(C) so only W[c,c,:,:] survives -> depthwise 3x3 "same" conv.

Strategy: channel-sharded across cores (32 ch/core, all 32 samples); the
whole conv runs on the PE as banded-Toeplitz matmuls:

  - partitions = (2 channels) x (64 H rows); per (pair, dw) the stationary
    operand is a 64x64 3-band Toeplitz block per channel, so a single matmul
    column computes the full 3-tap H-convolution for 2 channels at once
    (384 useful MACs/cycle vs 128 for the diagonal-weights scheme).
  - the two 64x64 blocks are issued as separate quadrant matmuls
    (tile_position from AP base partitions) so only the nonzero halves of
    the block-diagonal lhsT are stored/DMAed (0.75 MB instead of 1.5 MB);
    the PE runs both quadrants concurrently.
  - 3 matmul passes per channel pair (dw = 0,-1,+1); W-boundary handled by
    column-clipped rhs/out access patterns, H-boundary by the band structure.
  - free dim = (32 samples x 64 w) = 2048 f32 PSUM = 4 banks, chunked into
    bank-sized matmuls of N=512; weights are reused across all 32 samples so
    only 16 pairs x 3 dw lhsT loads per core.
  - wire dtypes sized to the 2e-2 rel-err budget: 16 samples fp16 + 16
    samples fp8 e4m3 (mixed-dtype matmuls vs fp16 weights; measured
    1.74e-2), output fp16; host does all layout/cast work.
  - eviction PSUM->SBUF casts to fp16, alternating ScalarE/VectorE per
    half-pair; input DMAs on the sync ring, output DMAs on the scalar ring;
    weights front-loaded in small chunks; last pair evicts per-chunk with
    dual-ring DMAs to shorten the end-of-program dependency chain.

Per-core traffic: 6.3 MB in + 8.4 MB out + 0.75 MB weights ~= 43 us at
358 GB/s; PE compute ~= 44 us; balanced at the ridge. HW exec ~= 61 us
(baseline diagonal-matmul/DVE fp32 kernel: 140.5 us).
"""

import os
from contextlib import ExitStack

import ml_dtypes
import numpy as np

import concourse.bass as bass
import concourse.tile as tile
from concourse import bacc, mybir
from concourse.bass_utils import run_bass_kernel_spmd

S, B, C, H, W_SP = 2, 16, 256, 64, 64
N_CORES = 8
NS = S * B                  # 32 samples (all on every core)
CPC = C // N_CORES          # 32 channels per core
NPAIR = CPC // 2            # 16 channel pairs per core
NCHUNK = 4                  # PSUM bank chunks per pair (512 f32 each)
SCH = NS // NCHUNK          # 8 samples per chunk
DWS = [0, -1, 1]            # dw=0 first: start=True must cover the full bank

F16 = mybir.dt.float16
F32 = mybir.dt.float32
F8 = mybir.dt.float8e4
F8_NP = ml_dtypes.float8_e4m3

# number of leading samples carried as fp8 (saves HBM read bytes; the
# rel-err budget of 2e-2 leaves room: measured 1.74e-2 at 16, 1.23e-2 at 8).
# fp8 samples go first: the small fp8 tile lands earliest, so the first
# MMs start sooner.
FP8_NS = int(os.environ.get("KERNEL_FP8_NS", "16"))
assert FP8_NS % SCH == 0
F16_NS = NS - FP8_NS


def _build_program():
    nc = bacc.Bacc("TRN2", target_bir_lowering=False, debug=False)
    x_d = nc.dram_tensor("x", [NPAIR * 128, F16_NS, W_SP], F16, kind="ExternalInput").ap()
    x8_d = None
    if FP8_NS:
        x8_d = nc.dram_tensor("x8", [NPAIR * 128, FP8_NS, W_SP], F8,
                              kind="ExternalInput").ap()
    wt_d = nc.dram_tensor("wt", [128, NPAIR * 3 * 64], F16, kind="ExternalInput").ap()
    out_d = nc.dram_tensor("out", [NPAIR * 128, NS * W_SP], F16, kind="ExternalOutput").ap()

    with tile.TileContext(nc) as tc:
        with ExitStack() as ctx:
            const_pool = ctx.enter_context(tc.tile_pool(name="const", bufs=1))
            wsb = const_pool.tile([128, NPAIR * 3 * 64], F16)

            xt_pool = ctx.enter_context(tc.tile_pool(name="xt", bufs=4))
            psum_pool = ctx.enter_context(tc.tile_pool(name="psum", bufs=4, space="PSUM"))
            osb_pool = ctx.enter_context(tc.tile_pool(name="osb", bufs=3))

            # weight chunks front-loaded: 1+3+4+4+4 pairs, all issued by k=3,
            # so pair 0 needs only ~180KB of transfers before its first MM
            wsl = 3 * 64
            wchunks = [(0, 1), (1, 3), (4, 4), (8, 4), (12, 4)]
            wc0, wcn = wchunks[0]
            nc.sync.dma_start(wsb[:, wc0 * wsl:(wc0 + wcn) * wsl],
                              wt_d[:, wc0 * wsl:(wc0 + wcn) * wsl])

            xts = []
            for k in range(NPAIR):
                r0, r1 = k * 128, (k + 1) * 128
                xt8 = None
                if FP8_NS:
                    xt8 = xt_pool.tile([128, FP8_NS, W_SP], F8, tag="xt8")
                    nc.sync.dma_start(xt8[:], x8_d[r0:r1, :, :])
                xt = xt_pool.tile([128, F16_NS, W_SP], F16, tag="xt")
                nc.sync.dma_start(xt[:], x_d[r0:r1, :, :])
                xts.append((xt, xt8))
                if 1 <= k <= len(wchunks) - 1:
                    wc0, wcn = wchunks[k]
                    nc.sync.dma_start(wsb[:, wc0 * wsl:(wc0 + wcn) * wsl],
                                      wt_d[:, wc0 * wsl:(wc0 + wcn) * wsl])

            for k in range(NPAIR):
                xt, xt8 = xts[k]
                r0, r1 = k * 128, (k + 1) * 128
                # two half-pair PSUM tiles (2 banks each) for finer pipelining
                pt_a = psum_pool.tile([128, 2, SCH, W_SP], F32, tag="pt")
                pt_b = psum_pool.tile([128, 2, SCH, W_SP], F32, tag="pt")
                pts = [pt_a, pt_b]
                for j, dw in enumerate(DWS):
                    wcol = (k * 3 + j) * 64
                    for q in range(NCHUNK):
                        pt = pts[q // 2]
                        s0 = q * SCH
                        if dw == 0:
                            ow = slice(0, W_SP)
                            iw = slice(0, W_SP)
                        elif dw == -1:
                            ow = slice(1, W_SP)
                            iw = slice(0, W_SP - 1)
                        else:
                            ow = slice(0, W_SP - 1)
                            iw = slice(1, W_SP)
                        if s0 < FP8_NS:
                            src, ss = xt8, s0
                        else:
                            src, ss = xt, s0 - FP8_NS
                        # block-diagonal lhsT: two 64x64 Toeplitz tiles, one
                        # matmul per PE quadrant-pair (tile_position inferred
                        # from AP base partitions); halves the weight bytes
                        for h in range(2):
                            p0, p1 = h * 64, h * 64 + 64
                            nc.tensor.matmul(
                                pt[p0:p1, q % 2, :, ow],
                                wsb[p0:p1, wcol:wcol + 64],
                                src[p0:p1, ss:ss + SCH, iw],
                                start=(j == 0), stop=(j == 2))

                ob = osb_pool.tile([128, NCHUNK * SCH * W_SP], F16, tag="ob")
                hfd = 2 * SCH * W_SP
                qfd = SCH * W_SP
                if k < NPAIR - 1:
                    for half in range(2):
                        dst = ob[:, half * hfd:(half + 1) * hfd]
                        if (k + half) % 2 == 0:
                            nc.scalar.copy(dst, pts[half][:, :, :, :])
                        else:
                            nc.vector.tensor_copy(dst, pts[half][:, :, :, :])
                    nc.scalar.dma_start(out_d[r0:r1, :], ob[:])
                else:
                    # last pair: fine-grained eviction + dual-ring out-DMAs
                    # shorten the end-of-program dependency chain
                    nc.scalar.copy(ob[:, 0:hfd], pts[0][:, :, :, :])
                    nc.scalar.dma_start(out_d[r0:r1, 0:hfd], ob[:, 0:hfd])
                    nc.scalar.copy(ob[:, hfd:hfd + qfd], pts[1][:, 0, :, :])
                    nc.vector.tensor_copy(ob[:, hfd + qfd:], pts[1][:, 1, :, :])
                    nc.scalar.dma_start(out_d[r0:r1, hfd:hfd + qfd],
                                        ob[:, hfd:hfd + qfd])
                    nc.sync.dma_start(out_d[r0:r1, hfd + qfd:],
                                      ob[:, hfd + qfd:])
    nc.compile()
    return nc


_prog_cache = {}


def _get_program():
    if FP8_NS not in _prog_cache:
        _prog_cache[FP8_NS] = _build_program()
    return _prog_cache[FP8_NS]


def _in_maps(x, W):
    wdiag = W[np.arange(C), np.arange(C)].astype(np.float32)   # [C,3,3]
    xs = x.reshape(NS, C, H, W_SP)
    eye = {d: np.eye(H, k=-d, dtype=np.float32) for d in (-1, 0, 1)}
    in_maps = []
    for core in range(N_CORES):
        c0 = core * CPC
        # x: [pair, (2ch x 64h) partitions, sample, w]
        A = xs[:, c0:c0 + CPC].transpose(1, 2, 0, 3)           # [32c, 64h, 32s, 64w]
        X = np.ascontiguousarray(
            A.reshape(2, NPAIR, H, NS, W_SP).transpose(1, 0, 2, 3, 4)
        ).reshape(NPAIR * 128, NS, W_SP)
        # weights: per (pair, dw, half) a 64x64 3-band Toeplitz block
        # T[p, o] = w[c, (p-o)+1, dw+1] for |p-o| <= 1, stacked so channel
        # c's block sits on partitions 0-63 and c+16's on 64-127
        wt = np.zeros((NPAIR, 3, 128, 64), dtype=np.float32)
        for k in range(NPAIR):
            for j, dw in enumerate(DWS):
                for half in range(2):
                    c = c0 + k + 16 * half
                    T = (wdiag[c, 0, dw + 1] * eye[-1]
                         + wdiag[c, 1, dw + 1] * eye[0]
                         + wdiag[c, 2, dw + 1] * eye[1])
                    h0 = 64 * half
                    wt[k, j, h0:h0 + 64, :] = T
        wt_host = np.ascontiguousarray(
            wt.transpose(2, 0, 1, 3)
        ).reshape(128, NPAIR * 3 * 64).astype(np.float16)
        m = {"x": np.ascontiguousarray(X[:, FP8_NS:]).astype(np.float16),
             "wt": wt_host}
        if FP8_NS:
            m["x8"] = np.ascontiguousarray(X[:, 0:FP8_NS]).astype(F8_NP)
        in_maps.append(m)
    return in_maps


def kernel(x: np.ndarray, W: np.ndarray) -> np.ndarray:
    x = np.ascontiguousarray(x, dtype=np.float32)
    W = np.ascontiguousarray(W, dtype=np.float32)
    assert x.shape == (S, B, C, H, W_SP)
    assert W.shape == (C, C, 3, 3)

    nc = _get_program()
    res = run_bass_kernel_spmd(nc, _in_maps(x, W), core_ids=list(range(N_CORES)))
    outs = []
    for core in range(N_CORES):
        oc = res.results[core]["out"].reshape(NPAIR, 2, H, NS, W_SP)
        outs.append(oc.transpose(3, 1, 0, 2, 4).reshape(NS, CPC, H, W_SP))
    out = np.concatenate(outs, axis=1).astype(np.float32)
    return out.reshape(S, B, C, H, W_SP)
